# revision 1
# baseline (speedup 1.0000x reference)
"""Trainium2 Bass kernel: windowed-std consistency loss.

Computes mean((local_std(argmax_c pred) - local_std(tgt))^2) with 5x5
zero-padded windows and unbiased (n-1) variance, n fixed at 25.

Architecture (per core, one batch element):
  stage A (elementwise): exact fp32 argmax via monotone prefix-max masks
      u_c = [prefix_max_c < max], A = sum(u_c), A^2 = Square(A);
      T = low-word(int64 target) -> bf16, T^2 = Square(T).
  pass 1 (PE): H-direction 5-box-sum via banded matmul with the *field* as
      lhsT, which transposes the tile for free: out[w, h'] = sum_h A[h,w] B[h,h'].
  evac: PSUM f32 -> SBUF bf16 (values are exact small ints).
  pass 2 (PE): W-direction 5-box-sum with the *band* as lhsT:
      out[w', h'] = sum_w B2[w,w'] Y[w,h'].
  var/std/mse: var = (s2 - s1^2/25)/24 (exact-int s1,s2), std = sqrt,
      accumulate (std_p - std_t)^2 per partition, reduce on host.
"""

import math
from contextlib import ExitStack

import ml_dtypes
import numpy as np

import concourse.bass as bass
import concourse.tile as tile
from concourse import mybir

F32 = mybir.dt.float32
BF16 = mybir.dt.bfloat16
I32 = mybir.dt.int32
AF = mybir.ActivationFunctionType
OP = mybir.AluOpType

JW = 124          # output rows/cols per filter window chunk (128 - 2*radius)
NCLS = 6
N_CORES = 8

# engine/strategy knobs
CFG = {
    "adds_via_dma": False,   # walrus birverifier rejects cce_op DMA copies
    "dsub_via_dma": False,
    "t_engine": "gpsimd",    # int32 (strided) -> bf16 convert for targets
    "cmp_engine": "vector",  # is_lt mask compares (Pool rejects compare/max TT ops)
    "adds_pool": True,       # half the mask-sum adds on Pool (supports add/sub only)
    "dsub_pool": False,      # d = std_p - std_t on DVE (Pool lengthens var chain)
    "gj": 3,                 # j-group size for pass2/var batching
    "evac_split": ("scalar", "scalar", "scalar", "scalar"),  # per field A,A2,T,T2
    "bufs": {
        "predp": 2, "tgtp": 2, "ptp": 7, "maskp": 8,
        "fieldp": 2, "yp": None, "vp": 8, "p1p": 2, "p2p": 4,
    },
}


def _band_np(kind, h_tail, w_tail):
    """128x128 bf16 band blocks; cols >= JW (or tail) forced to zero."""
    r = np.arange(128)[:, None]
    c = np.arange(128)[None, :]
    if kind == "edge":       # first window: rows are absolute image rows
        m = np.abs(r - c) <= 2
        lim = JW
    elif kind == "int":      # interior: window starts 2 rows before outputs
        m = (r - c >= 0) & (r - c <= 4)
        lim = JW
    else:                    # interior band, truncated for the last window
        m = (r - c >= 0) & (r - c <= 4)
        lim = h_tail if kind == "last_h" else w_tail
    b = np.where(m, 1.0, 0.0).astype(np.float32)
    b[:, lim:] = 0.0
    return b.astype(ml_dtypes.bfloat16)


def _windows(n):
    """List of (r0, plen, out0, outn) for JW-strided 5-wide box windows."""
    nj = -(-n // JW)
    out = []
    for j in range(nj):
        o0 = JW * j
        on = min(JW, n - o0)
        r0 = max(0, o0 - 2)
        plen = min(128, n - r0)
        out.append((r0, plen, o0, on))
    return out


def build_nc(H, W, tgt_pairs=True, legalize=True, reps=1):
    assert H % 4 == 0 and W % 4 == 0
    nc = bass.Bass()
    pred = nc.dram_tensor("pred", [NCLS, H, W], F32, kind="ExternalInput")
    tgt = nc.dram_tensor("tgt", [H, (2 if tgt_pairs else 1) * W], I32, kind="ExternalInput")
    out_d = nc.dram_tensor("out", [128, 1], F32, kind="ExternalOutput")

    jwin = _windows(H)
    kwin = _windows(W)
    NJ, NK = len(jwin), len(kwin)
    h_tail = jwin[-1][3]
    w_tail = kwin[-1][3]
    NKA = min(NK - 1, 8)      # full-height blocks packed into main psum tile
    for _k in range(NKA):
        assert kwin[_k][1] == 128, f"non-128 interior w-block {_k}: {kwin[_k]}"
    YW = 128 * NK             # yrow width (128-strided blocks)
    GJ = CFG["gj"]
    NG = -(-NJ // GJ)
    assert NG * NK <= 64

    band_e_d = nc.inline_tensor(np.asarray(_band_np("edge", h_tail, w_tail)), "band_e")
    band_i_d = nc.inline_tensor(np.asarray(_band_np("int", h_tail, w_tail)), "band_i")
    band_lh_d = nc.inline_tensor(np.asarray(_band_np("last_h", h_tail, w_tail)), "band_lh")

    cmp_e = {"gpsimd": nc.gpsimd, "vector": nc.vector}[CFG["cmp_engine"]]
    evac_e = {"vector": nc.vector, "scalar": None}

    with tile.TileContext(nc) as tc, ExitStack() as ctx:
        bf = CFG["bufs"]
        yp_bufs = bf["yp"] if bf["yp"] else 4 * (GJ + 1)
        const = ctx.enter_context(tc.tile_pool(name="const", bufs=1))
        predp = ctx.enter_context(tc.tile_pool(name="predp", bufs=bf["predp"]))
        tgtp = ctx.enter_context(tc.tile_pool(name="tgtp", bufs=bf["tgtp"]))
        ptp = ctx.enter_context(tc.tile_pool(name="ptp", bufs=bf["ptp"]))
        maskp = ctx.enter_context(tc.tile_pool(name="maskp", bufs=bf["maskp"]))
        fieldp = ctx.enter_context(tc.tile_pool(name="fieldp", bufs=bf["fieldp"]))
        yp = ctx.enter_context(tc.tile_pool(name="yp", bufs=yp_bufs))
        vp = ctx.enter_context(tc.tile_pool(name="vp", bufs=bf["vp"]))
        p1p = ctx.enter_context(tc.tile_pool(name="p1p", bufs=bf["p1p"], space="PSUM"))
        p2p = ctx.enter_context(tc.tile_pool(name="p2p", bufs=bf["p2p"], space="PSUM"))

        b_e = const.tile([128, 128], BF16, tag="be")
        nc.gpsimd.dma_start(b_e[:], band_e_d.ap())
        b_i = const.tile([128, 128], BF16, tag="bi")
        nc.gpsimd.dma_start(b_i[:], band_i_d.ap())
        b_lh = const.tile([128, 128], BF16, tag="blh")
        nc.gpsimd.dma_start(b_lh[:], band_lh_d.ap())

        acc = const.tile([128, 64], F32, tag="acc")
        nc.vector.memset(acc[:], 0.0)

        yrows = {}  # (f, j) -> sbuf tile [128, YW]

        def stage_a(j):
            r0, plen, _, _ = jwin[j]
            # Two 3-plane DMAs: each prefix-max then depends on at most one
            # DMA queue semaphore (this walrus allows 1 sync wait per inst).
            xa = predp.tile([128, 3, W], F32, tag="xa")
            nc.gpsimd.dma_start(
                xa[0:plen],
                pred.ap()[0:3, r0 : r0 + plen, :].rearrange("c p w -> p c w"),
            )
            xb = predp.tile([128, 3, W], F32, tag="xb")
            nc.gpsimd.dma_start(
                xb[0:plen],
                pred.ap()[3:6, r0 : r0 + plen, :].rearrange("c p w -> p c w"),
            )
            xs = [xa[:, 0, :], xa[:, 1, :], xa[:, 2, :], xb[:, 0, :], xb[:, 1, :], xb[:, 2, :]]
            tg = tgtp.tile([128, (2 if tgt_pairs else 1) * W], I32, tag="tg")
            nc.gpsimd.dma_start(tg[0:plen, :], tgt.ap()[r0 : r0 + plen, :])

            nh = CFG.get("stage_a_splits", 1)
            halves = [(i * (W // nh), W // nh) for i in range(nh)]
            pth = {}
            for hi, (h0, hw) in enumerate(halves):
                prev = xs[0][:, h0 : h0 + hw]
                row = []
                for c in range(1, NCLS):
                    pc = ptp.tile([128, W // nh], F32, tag=f"pt{hi}")
                    nc.vector.tensor_max(
                        pc[0:plen, :], prev[0:plen, :],
                        xs[c][0:plen, h0 : h0 + hw],
                    )
                    row.append(pc)
                    prev = pc[:, :]
                pth[hi] = row

            fA = fieldp.tile([128, W], BF16, tag="fA")
            add2_e = nc.gpsimd if CFG["adds_pool"] else nc.vector
            for hi, (h0, hw) in enumerate(halves):
                pts = pth[hi]
                m = pts[-1]
                us = []
                for c in range(0, NCLS - 1):
                    u = maskp.tile([128, W // nh], BF16, tag=f"u{hi}")
                    src = xs[0][:, h0 : h0 + hw] if c == 0 else pts[c - 1][:, :]
                    cmp_e.tensor_tensor(
                        u[0:plen, :], src[0:plen, :], m[0:plen, :], op=OP.is_lt
                    )
                    us.append(u)
                a1 = maskp.tile([128, W // nh], BF16, tag=f"u{hi}")
                nc.vector.tensor_add(a1[0:plen, :], us[0][0:plen, :], us[1][0:plen, :])
                a2 = maskp.tile([128, W // nh], BF16, tag=f"u{hi}")
                add2_e.tensor_add(a2[0:plen, :], us[2][0:plen, :], us[3][0:plen, :])
                a3 = maskp.tile([128, W // nh], BF16, tag=f"u{hi}")
                add2_e.tensor_add(a3[0:plen, :], a1[0:plen, :], us[4][0:plen, :])
                nc.vector.tensor_add(
                    fA[0:plen, h0 : h0 + hw], a3[0:plen, :], a2[0:plen, :]
                )

            fA2 = fieldp.tile([128, W], BF16, tag="fA2")
            nc.scalar.activation(fA2[0:plen, :], fA[0:plen, :], AF.Square)

            fT = fieldp.tile([128, W], BF16, tag="fT")
            if tgt_pairs:
                tg_lo = tg[0:plen, :].rearrange("p (w two) -> p w two", two=2)[:, :, 0]
            else:
                tg_lo = tg[0:plen, :]
            if CFG["t_engine"] == "gpsimd":
                nc.gpsimd.tensor_copy(fT[0:plen, :], tg_lo)
            else:
                nc.vector.tensor_copy(fT[0:plen, :], tg_lo)
            fT2 = fieldp.tile([128, W], BF16, tag="fT2")
            nc.scalar.activation(fT2[0:plen, :], fT[0:plen, :], AF.Square)
            return [fA, fA2, fT, fT2]

        def pass1(j, fields):
            r0, plen, _, _ = jwin[j]
            band = b_e if j == 0 else (b_lh if j == NJ - 1 else b_i)
            for fi, f in enumerate(fields):
                psA = p1p.tile([128, 128 * NKA], F32, tag="ps1")
                for k in range(NKA):
                    wc0, mk = kwin[k][0], kwin[k][1]
                    nc.tensor.matmul(
                        psA[0:mk, 128 * k : 128 * (k + 1)],
                        f[0:plen, wc0 : wc0 + mk],
                        band[0:plen, :],
                        start=True,
                        stop=True,
                    )
                y = yp.tile([128, YW], BF16, tag="y")
                eng = CFG["evac_split"][fi]
                if eng == "vector":
                    nc.vector.tensor_copy(y[:, 0 : 128 * NKA], psA[:])
                else:
                    nc.scalar.copy(y[:, 0 : 128 * NKA], psA[:])
                for k in range(NKA, NK):
                    psB = p1p.tile([128, 128], F32, tag="ps1")
                    wc0, mk = kwin[k][0], kwin[k][1]
                    nc.tensor.matmul(
                        psB[0:mk, :],
                        f[0:plen, wc0 : wc0 + mk],
                        band[0:plen, :],
                        start=True,
                        stop=True,
                    )
                    if eng == "vector":
                        nc.vector.tensor_copy(
                            y[0:mk, 128 * k : 128 * (k + 1)], psB[0:mk, :]
                        )
                    else:
                        nc.scalar.copy(y[0:mk, 128 * k : 128 * (k + 1)], psB[0:mk, :])
                yrows[(fi, j)] = y

        def pass2_var(g):
            js = list(range(g * GJ, min((g + 1) * GJ, NJ)))
            gw = 128 * len(js)
            pair_ps = GJ <= 2  # two stats share one PSUM bank (512 f32)
            for k in range(NK):
                kmk = kwin[k][1]
                m2 = kwin[k][3]
                band2 = b_e if k == 0 else b_i
                ps2 = []
                if pair_ps:
                    ppa = p2p.tile([128, 512], F32, tag="ps2")
                    ppb = p2p.tile([128, 512], F32, tag="ps2")
                    tiles = [ppa, ppb]
                    for fi in range(4):
                        p = tiles[fi // 2]
                        off = 256 * (fi % 2)
                        for idx, j in enumerate(js):
                            nc.tensor.matmul(
                                p[0:m2, off + 128 * idx : off + 128 * (idx + 1)],
                                band2[0:kmk, 0:m2],
                                yrows[(fi, j)][0:kmk, 128 * k : 128 * (k + 1)],
                                start=True,
                                stop=True,
                            )
                        ps2.append(p[:, off : off + gw])
                else:
                    for fi in range(4):
                        p = p2p.tile([128, 128 * GJ], F32, tag="ps2")
                        for idx, j in enumerate(js):
                            nc.tensor.matmul(
                                p[0:m2, 128 * idx : 128 * (idx + 1)],
                                band2[0:kmk, 0:m2],
                                yrows[(fi, j)][0:kmk, 128 * k : 128 * (k + 1)],
                                start=True,
                                stop=True,
                            )
                        ps2.append(p[:, 0:gw])

                sA = vp.tile([128, gw], F32, tag="v")
                nc.scalar.activation(sA[0:m2, :], ps2[0][0:m2, :], AF.Square)
                vA = vp.tile([128, gw], F32, tag="v")
                nc.vector.scalar_tensor_tensor(
                    vA[0:m2, :], sA[0:m2, :], 1.0 / 25.0, ps2[1][0:m2, :],
                    op0=OP.mult, op1=OP.subtract,
                )
                stA = vp.tile([128, gw], F32, tag="v")
                nc.scalar.activation(
                    stA[0:m2, :], vA[0:m2, :], AF.Sqrt, scale=-1.0 / 24.0
                )
                sT = vp.tile([128, gw], F32, tag="v")
                nc.scalar.activation(sT[0:m2, :], ps2[2][0:m2, 0:gw], AF.Square)
                vT = vp.tile([128, gw], F32, tag="v")
                nc.vector.scalar_tensor_tensor(
                    vT[0:m2, :], sT[0:m2, :], 1.0 / 25.0, ps2[3][0:m2, 0:gw],
                    op0=OP.mult, op1=OP.subtract,
                )
                stT = vp.tile([128, gw], F32, tag="v")
                nc.scalar.activation(
                    stT[0:m2, :], vT[0:m2, :], AF.Sqrt, scale=-1.0 / 24.0
                )
                slot = g * NK + k
                if CFG["dsub_via_dma"]:
                    nc.gpsimd.dma_start(
                        stT[0:m2, :], stA[0:m2, :], accum_op=OP.subtract
                    )
                    dtile = stT
                else:
                    dsub_e = nc.gpsimd if CFG["dsub_pool"] else nc.vector
                    dtile = vp.tile([128, gw], F32, tag="v")
                    dsub_e.tensor_sub(dtile[0:m2, :], stA[0:m2, :], stT[0:m2, :])
                dsq = vp.tile([128, gw], F32, tag="v")
                nc.scalar.activation(
                    dsq[0:m2, :], dtile[0:m2, :], AF.Square,
                    accum_out=acc[0:m2, slot : slot + 1],
                )
            for j in js:
                for fi in range(4):
                    del yrows[(fi, j)]

        for _rep in range(reps):
            for j in range(NJ):
                fields = stage_a(j)
                pass1(j, fields)
                if j % GJ == GJ - 1 or j == NJ - 1:
                    pass2_var(j // GJ)

        accsum = const.tile([128, 1], F32, tag="accsum")
        nc.vector.reduce_sum(accsum[:], acc[:], axis=mybir.AxisListType.X)
        nc.gpsimd.dma_start(out_d.ap(), accsum[:])

    return _legalize_sync_waits(nc) if legalize else nc


def _legalize_sync_waits(nc):
    """This walrus build allows at most 2 sync commands (waits + updates)
    per instruction. Tile emits more on multi-dependency instructions and on
    its kernel-tail drain. Split excess waits onto preceding same-engine
    NoOps (sequencer-executed, so the engine still blocks before the
    original instruction issues — semantics preserved)."""
    import bass_rust

    nid = 0
    for fn in nc.m.functions:
        for blk in fn.blocks:
            il = blk.instructions
            out = []
            changed = False
            for ins in il:
                si = ins.sync_info
                if si is not None:
                    waits = list(si.on_wait)
                    upds = list(si.on_update)
                    keep = 1
                    if len(waits) > keep:
                        extra = waits[: len(waits) - keep]
                        kept = waits[len(waits) - keep :]
                        for w in extra:
                            nop = mybir.InstNoOp(
                                name=f"syncsplit-{nid}", ins=[], outs=[]
                            )
                            nid += 1
                            nop.engine = ins.engine
                            nop.sync_info = bass_rust.SyncInfo(
                                on_wait=[w], on_update=[]
                            )
                            out.append(nop)
                        ins.sync_info = bass_rust.SyncInfo(
                            on_wait=kept, on_update=upds
                        )
                        changed = True
                out.append(ins)
            if changed:
                il.clear()
                il.extend(out)
    return nc


_NC_CACHE = {}


def _get_nc(H, W, tgt_pairs=True):
    key = (H, W, tgt_pairs, str(sorted(CFG.items())))
    if key not in _NC_CACHE:
        _NC_CACHE[key] = build_nc(H, W, tgt_pairs=tgt_pairs)
    return _NC_CACHE[key]


def make_in_maps(predictions, targets):
    B, C, H, W = predictions.shape
    pairs = targets.dtype == np.int64
    in_maps = []
    for b in range(B):
        t = np.ascontiguousarray(targets[b])
        t = t.view(np.int32).reshape(H, 2 * W) if pairs else t.astype(np.int32)
        in_maps.append({"pred": np.ascontiguousarray(predictions[b]), "tgt": t})
    return in_maps


def kernel(predictions, targets, kernel_size):
    from concourse.bass_utils import run_bass_kernel_spmd

    assert int(kernel_size) == 5
    predictions = np.asarray(predictions)
    targets = np.asarray(targets)
    B, C, H, W = predictions.shape
    assert C == NCLS and B == N_CORES
    assert targets.dtype in (np.int64, np.int32)

    nc = _get_nc(H, W, tgt_pairs=targets.dtype == np.int64)
    in_maps = make_in_maps(predictions, targets)
    res = run_bass_kernel_spmd(nc, in_maps, core_ids=list(range(N_CORES)))
    total = 0.0
    for r in res.results:
        total += float(r["out"].astype(np.float64).sum())
    return np.float32(total / (B * H * W))



# revision 10
# speedup vs baseline: 391.6026x; 391.6026x over previous
"""Trainium2 Bass kernel: windowed-std consistency loss.

Computes mean((local_std(argmax_c pred) - local_std(tgt))^2) with 5x5
zero-padded windows and unbiased (n-1) variance, n fixed at 25.

Per core (one batch element), two interleaved phases over 8 aligned
128-row tiles:

  phase 1 (per tile): DMA 6 class planes (f32, contiguous 4KB lines) +
      int8 targets; exact argmax A via monotone prefix-max masks
      (5 max + 5 is_lt on DVE); fields A, A^2, T, T^2 as zero-padded
      [128, W+4] bf16 tiles; W-direction 5-box-sum via 4 shifted bf16
      adds per field on Pool (partial sums <= 125, exact in bf16).
  phase 2 (per tile, lagged by 1): H-direction 5-box-sum on PE — per
      field a banded [128,128] matmul plus zero-padded seam bands that
      accumulate the +-2-row contributions from the adjacent tiles'
      W-sums into the same PSUM group (start/stop accumulation), so
      tiles stay 128-aligned and nothing is re-fetched or recomputed.
      var/std straight out of PSUM: s1q = Square(0.2*s1) [Act],
      v = s1q - s2 [DVE] (= -24 var), std = Sqrt(-v/24) [Act],
      d = stdA - stdT [DVE], Square+accum into acc column [Act].

Targets ship as int8 (values 0..5; lossless cast from int64/int32 on
host), cutting per-core input bytes from 32MB to 25MB.
"""

from contextlib import ExitStack

import ml_dtypes
import numpy as np

import concourse.bass as bass
import concourse.tile as tile
from concourse import mybir

F32 = mybir.dt.float32
BF16 = mybir.dt.bfloat16
I8 = mybir.dt.int8
AF = mybir.ActivationFunctionType
OP = mybir.AluOpType

NCLS = 6
N_CORES = 8
TP = 128  # tile rows (partition dim)


def _bands_np():
    """Main/prev/next H-direction 5-band blocks, bf16 [128,128].

    out[h'] = sum_h band[h, h'] * x[h]; main covers |h-h'|<=2 within the
    tile, prev/next cover the 2-row halos from the adjacent tiles.
    """
    r = np.arange(TP)[:, None]
    c = np.arange(TP)[None, :]
    main = (np.abs(r - c) <= 2).astype(np.float32)
    prev = ((r >= TP - 2) & (np.abs((r - TP) - c) <= 2)).astype(np.float32)
    nxt = ((r <= 1) & (np.abs((r + TP) - c) <= 2)).astype(np.float32)
    return [b.astype(ml_dtypes.bfloat16) for b in (main, prev, nxt)]


def build_nc(H, W, reps=1, legalize=True):
    assert H % TP == 0
    NT = H // TP
    nc = bass.Bass()
    pred = nc.dram_tensor("pred", [NCLS, H, W], F32, kind="ExternalInput")
    tgt = nc.dram_tensor("tgt", [H, W], I8, kind="ExternalInput")
    out_d = nc.dram_tensor("out", [128, 1], F32, kind="ExternalOutput")

    bm_np, bp_np, bn_np = _bands_np()
    bm_d = nc.inline_tensor(np.ascontiguousarray(bm_np), "band_m")
    bp_d = nc.inline_tensor(np.ascontiguousarray(bp_np), "band_p")
    bn_d = nc.inline_tensor(np.ascontiguousarray(bn_np), "band_n")

    MMW = 512  # matmul free-dim chunk (one PSUM bank of f32)
    NHALF = W // MMW

    with tile.TileContext(nc) as tc, ExitStack() as ctx:
        const = ctx.enter_context(tc.tile_pool(name="const", bufs=1))
        predp = ctx.enter_context(tc.tile_pool(name="predp", bufs=3))
        tgtp = ctx.enter_context(tc.tile_pool(name="tgtp", bufs=2))
        ptp = ctx.enter_context(tc.tile_pool(name="ptp", bufs=5))
        maskp = ctx.enter_context(tc.tile_pool(name="maskp", bufs=6))
        fieldp = ctx.enter_context(tc.tile_pool(name="fieldp", bufs=6))
        wtmpp = ctx.enter_context(tc.tile_pool(name="wtmpp", bufs=4))
        wsump = ctx.enter_context(tc.tile_pool(name="wsump", bufs=12))
        stdp = ctx.enter_context(tc.tile_pool(name="stdp", bufs=4))
        psump = ctx.enter_context(tc.tile_pool(name="psump", bufs=8, space="PSUM"))

        b_m = const.tile([TP, TP], BF16, tag="bm")
        nc.sync.dma_start(b_m[:], bm_d.ap())
        b_p = const.tile([TP, TP], BF16, tag="bp")
        nc.sync.dma_start(b_p[:], bp_d.ap())
        b_n = const.tile([TP, TP], BF16, tag="bn")
        nc.sync.dma_start(b_n[:], bn_d.ap())

        acc = const.tile([128, 2 * NT], F32, tag="acc")
        nc.vector.memset(acc[:], 0.0)

        wsums = {}  # (field, tile_j) -> [128, W] bf16 W-direction box sums

        def phase1(j):
            r0 = TP * j
            xa = predp.tile([TP, 3, W], F32, tag="xa")
            nc.sync.dma_start(
                xa[:], pred.ap()[0:3, r0 : r0 + TP, :].rearrange("c p w -> p c w")
            )
            xb = predp.tile([TP, 3, W], F32, tag="xb")
            nc.scalar.dma_start(
                xb[:], pred.ap()[3:6, r0 : r0 + TP, :].rearrange("c p w -> p c w")
            )
            tg = tgtp.tile([TP, W], I8, tag="tg")
            nc.sync.dma_start(tg[:], tgt.ap()[r0 : r0 + TP, :])

            xs = [xa[:, 0, :], xa[:, 1, :], xa[:, 2, :],
                  xb[:, 0, :], xb[:, 1, :], xb[:, 2, :]]
            pts = []
            prev = xs[0]
            for c in range(1, NCLS):
                pc = ptp.tile([TP, W], F32, tag="pt")
                nc.vector.tensor_max(pc[:], prev, xs[c])
                pts.append(pc)
                prev = pc[:, :]
            m = pts[-1]
            us = []
            for c in range(NCLS - 1):
                u = maskp.tile([TP, W], BF16, tag="u")
                src = xs[0] if c == 0 else pts[c - 1][:, :]
                nc.vector.tensor_tensor(u[:], src, m[:, :], op=OP.is_lt)
                us.append(u)

            fA = fieldp.tile([TP, W + 4], BF16, tag="fA")
            nc.vector.memset(fA[:, 0:2], 0.0)
            nc.vector.memset(fA[:, W + 2 : W + 4], 0.0)
            a1 = maskp.tile([TP, W], BF16, tag="u")
            nc.vector.tensor_add(a1[:], us[0][:, :], us[1][:, :])
            a2 = maskp.tile([TP, W], BF16, tag="u")
            nc.gpsimd.tensor_add(a2[:], us[2][:, :], us[3][:, :])
            a3 = maskp.tile([TP, W], BF16, tag="u")
            nc.vector.tensor_add(a3[:], a1[:, :], a2[:, :])
            nc.gpsimd.tensor_add(fA[:, 2 : W + 2], a3[:, :], us[4][:, :])

            fA2 = fieldp.tile([TP, W + 4], BF16, tag="fA2")
            nc.scalar.activation(fA2[:], fA[:, :], AF.Square)

            fT = fieldp.tile([TP, W + 4], BF16, tag="fT")
            nc.vector.memset(fT[:, 0:2], 0.0)
            nc.vector.memset(fT[:, W + 2 : W + 4], 0.0)
            nc.gpsimd.tensor_copy(fT[:, 2 : W + 2], tg[:, :])
            fT2 = fieldp.tile([TP, W + 4], BF16, tag="fT2")
            nc.scalar.activation(fT2[:], fT[:, :], AF.Square)

            engs = {
                0: (nc.vector, nc.vector, nc.vector, nc.vector),
                1: (nc.vector, nc.vector, nc.vector, nc.vector),
                2: (nc.vector, nc.vector, nc.vector, nc.gpsimd),
                3: (nc.gpsimd, nc.gpsimd, nc.gpsimd, nc.gpsimd),
            }
            for fi, f in enumerate((fA, fA2, fT, fT2)):
                e1, e2, e3, e4 = engs[fi]
                t1 = wtmpp.tile([TP, W], BF16, tag="wt")
                e1.tensor_add(t1[:], f[:, 0:W], f[:, 1 : W + 1])
                t2 = wtmpp.tile([TP, W], BF16, tag="wt")
                e2.tensor_add(t2[:], f[:, 2 : W + 2], f[:, 3 : W + 3])
                t3 = wtmpp.tile([TP, W], BF16, tag="wt")
                e3.tensor_add(t3[:], t1[:, :], t2[:, :])
                ws = wsump.tile([TP, W], BF16, tag="ws")
                e4.tensor_add(ws[:], t3[:, :], f[:, 4 : W + 4])
                wsums[(fi, j)] = ws

        def phase2(j):
            for h in range(NHALF):
                sl = slice(MMW * h, MMW * (h + 1))
                ps = []
                for fi in range(4):
                    p = psump.tile([TP, MMW], F32, tag="ps")
                    nc.tensor.matmul(
                        p[:], b_m[:], wsums[(fi, j)][:, sl],
                        start=True, stop=j == 0 and j == NT - 1,
                    )
                    if j > 0:
                        nc.tensor.matmul(
                            p[:], b_p[:], wsums[(fi, j - 1)][:, sl],
                            start=False, stop=j == NT - 1,
                        )
                    if j < NT - 1:
                        nc.tensor.matmul(
                            p[:], b_n[:], wsums[(fi, j + 1)][:, sl],
                            start=False, stop=True,
                        )
                    ps.append(p)
                stds = []
                for s1, s2 in ((ps[0], ps[1]), (ps[2], ps[3])):
                    sq = stdp.tile([TP, MMW], F32, tag="sd")
                    nc.scalar.activation(sq[:], s1[:, :], AF.Square, scale=0.2)
                    v = stdp.tile([TP, MMW], F32, tag="sd")
                    nc.vector.tensor_sub(v[:], sq[:, :], s2[:, :])
                    st = stdp.tile([TP, MMW], F32, tag="sd")
                    nc.scalar.activation(st[:], v[:, :], AF.Sqrt, scale=-1.0 / 24.0)
                    stds.append(st)
                d = stdp.tile([TP, MMW], F32, tag="sd")
                nc.vector.tensor_sub(d[:], stds[0][:, :], stds[1][:, :])
                dsq = stdp.tile([TP, MMW], F32, tag="sd")
                slot = 2 * j + h
                nc.scalar.activation(
                    dsq[:], d[:, :], AF.Square, accum_out=acc[:, slot : slot + 1]
                )
            for fi in range(4):
                wsums.pop((fi, j - 1), None)

        for _rep in range(reps):
            for j in range(NT):
                phase1(j)
                if j >= 1:
                    phase2(j - 1)
            phase2(NT - 1)

        accsum = const.tile([128, 1], F32, tag="accsum")
        nc.vector.reduce_sum(accsum[:], acc[:], axis=mybir.AxisListType.X)
        nc.sync.dma_start(out_d.ap(), accsum[:])

    return _legalize_sync_waits(nc) if legalize else nc


def _legalize_sync_waits(nc):
    """This walrus build allows at most 2 sync commands (waits + updates)
    per instruction. Tile emits more on multi-dependency instructions and on
    its kernel-tail drain. Split excess waits onto preceding same-engine
    NoOps (sequencer-executed, so the engine still blocks before the
    original instruction issues — semantics preserved)."""
    import bass_rust

    nid = 0
    for fn in nc.m.functions:
        for blk in fn.blocks:
            il = blk.instructions
            out = []
            changed = False
            for ins in il:
                si = ins.sync_info
                if si is not None:
                    waits = list(si.on_wait)
                    upds = list(si.on_update)
                    keep = 1
                    if len(waits) > keep:
                        extra = waits[: len(waits) - keep]
                        kept = waits[len(waits) - keep :]
                        for w in extra:
                            nop = mybir.InstNoOp(
                                name=f"syncsplit-{nid}", ins=[], outs=[]
                            )
                            nid += 1
                            nop.engine = ins.engine
                            nop.sync_info = bass_rust.SyncInfo(
                                on_wait=[w], on_update=[]
                            )
                            out.append(nop)
                        ins.sync_info = bass_rust.SyncInfo(
                            on_wait=kept, on_update=upds
                        )
                        changed = True
                out.append(ins)
            if changed:
                il.clear()
                il.extend(out)
    return nc


_NC_CACHE = {}


def _get_nc(H, W, reps=1):
    key = (H, W, reps)
    if key not in _NC_CACHE:
        _NC_CACHE[key] = build_nc(H, W, reps=reps)
    return _NC_CACHE[key]


def make_in_maps(predictions, targets):
    B = predictions.shape[0]
    t8 = np.ascontiguousarray(targets.astype(np.int8))
    return [
        {"pred": np.ascontiguousarray(predictions[b]), "tgt": t8[b]}
        for b in range(B)
    ]


def kernel(predictions, targets, kernel_size):
    from concourse.bass_utils import run_bass_kernel_spmd

    assert int(kernel_size) == 5
    predictions = np.asarray(predictions)
    targets = np.asarray(targets)
    B, C, H, W = predictions.shape
    assert C == NCLS and B == N_CORES

    nc = _get_nc(H, W)
    in_maps = make_in_maps(predictions, targets)
    res = run_bass_kernel_spmd(nc, in_maps, core_ids=list(range(N_CORES)))
    total = 0.0
    for r in res.results:
        total += float(r["out"].astype(np.float64).sum())
    return np.float32(total / (B * H * W))


# revision 11
# speedup vs baseline: 455.9015x; 1.1642x over previous
"""Trainium2 Bass kernel: windowed-std consistency loss.

Computes mean((local_std(argmax_c pred) - local_std(tgt))^2) with 5x5
zero-padded windows and unbiased (n-1) variance, n fixed at 25.

Per core (one batch element), two interleaved phases over 8 aligned
128-row tiles:

  phase 1 (per tile): DMA 6 class planes (f32, contiguous 4KB lines) +
      int8 targets; exact argmax A via monotone prefix-max masks
      (5 max + 5 is_lt on DVE); fields A, A^2, T, T^2 as zero-padded
      [128, W+4] bf16 tiles; W-direction 5-box-sum via 4 shifted
      in-place bf16 accumulating adds per field on DVE (partial sums
      <= 125, exact in bf16; in-place is safe, same element offsets).
  phase 2 (per tile, lagged by 1): H-direction 5-box-sum on PE — per
      field a banded [128,128] matmul plus zero-padded seam bands that
      accumulate the +-2-row contributions from the adjacent tiles'
      W-sums into the same PSUM group (start/stop accumulation), so
      tiles stay 128-aligned and nothing is re-fetched or recomputed.
      var/std straight out of PSUM: s1q = Square(0.2*s1) [Act],
      v = s1q - s2 [DVE] (= -24 var), std = Sqrt(-v/24) [Act],
      d = stdA - stdT [DVE], Square+accum into acc column [Act].

Targets ship as int8 (values 0..5; lossless cast from int64/int32 on
host), cutting per-core input bytes from 32MB to 25MB.
"""

from contextlib import ExitStack

import ml_dtypes
import numpy as np

import concourse.bass as bass
import concourse.tile as tile
from concourse import mybir

F32 = mybir.dt.float32
BF16 = mybir.dt.bfloat16
I8 = mybir.dt.int8
AF = mybir.ActivationFunctionType
OP = mybir.AluOpType

NCLS = 6
N_CORES = 8
TP = 128  # tile rows (partition dim)


def _bands_np():
    """Main/prev/next H-direction 5-band blocks, bf16 [128,128].

    out[h'] = sum_h band[h, h'] * x[h]; main covers |h-h'|<=2 within the
    tile, prev/next cover the 2-row halos from the adjacent tiles.
    """
    r = np.arange(TP)[:, None]
    c = np.arange(TP)[None, :]
    main = (np.abs(r - c) <= 2).astype(np.float32)
    prev = ((r >= TP - 2) & (np.abs((r - TP) - c) <= 2)).astype(np.float32)
    nxt = ((r <= 1) & (np.abs((r + TP) - c) <= 2)).astype(np.float32)
    return [b.astype(ml_dtypes.bfloat16) for b in (main, prev, nxt)]


def build_nc(H, W, reps=1, legalize=True):
    assert H % TP == 0
    NT = H // TP
    nc = bass.Bass()
    pred = nc.dram_tensor("pred", [NCLS, H, W], F32, kind="ExternalInput")
    tgt = nc.dram_tensor("tgt", [H, W], I8, kind="ExternalInput")
    out_d = nc.dram_tensor("out", [128, 1], F32, kind="ExternalOutput")

    bm_np, bp_np, bn_np = _bands_np()
    bm_d = nc.inline_tensor(np.ascontiguousarray(bm_np), "band_m")
    bp_d = nc.inline_tensor(np.ascontiguousarray(bp_np), "band_p")
    bn_d = nc.inline_tensor(np.ascontiguousarray(bn_np), "band_n")

    MMW = 512  # matmul free-dim chunk (one PSUM bank of f32)
    NHALF = W // MMW

    with tile.TileContext(nc) as tc, ExitStack() as ctx:
        const = ctx.enter_context(tc.tile_pool(name="const", bufs=1))
        predp = ctx.enter_context(tc.tile_pool(name="predp", bufs=3))
        tgtp = ctx.enter_context(tc.tile_pool(name="tgtp", bufs=2))
        ptp = ctx.enter_context(tc.tile_pool(name="ptp", bufs=5))
        maskp = ctx.enter_context(tc.tile_pool(name="maskp", bufs=6))
        fieldp = ctx.enter_context(tc.tile_pool(name="fieldp", bufs=6))
        wsump = ctx.enter_context(tc.tile_pool(name="wsump", bufs=12))
        stdp = ctx.enter_context(tc.tile_pool(name="stdp", bufs=6))
        psump = ctx.enter_context(tc.tile_pool(name="psump", bufs=8, space="PSUM"))

        b_m = const.tile([TP, TP], BF16, tag="bm")
        nc.sync.dma_start(b_m[:], bm_d.ap())
        b_p = const.tile([TP, TP], BF16, tag="bp")
        nc.sync.dma_start(b_p[:], bp_d.ap())
        b_n = const.tile([TP, TP], BF16, tag="bn")
        nc.sync.dma_start(b_n[:], bn_d.ap())

        acc = const.tile([128, 2 * NT], F32, tag="acc")
        nc.vector.memset(acc[:], 0.0)

        wsums = {}  # (field, tile_j) -> [128, W] bf16 W-direction box sums

        def phase1(j):
            r0 = TP * j
            xa = predp.tile([TP, 3, W], F32, tag="xa")
            nc.sync.dma_start(
                xa[:], pred.ap()[0:3, r0 : r0 + TP, :].rearrange("c p w -> p c w")
            )
            xb = predp.tile([TP, 3, W], F32, tag="xb")
            nc.scalar.dma_start(
                xb[:], pred.ap()[3:6, r0 : r0 + TP, :].rearrange("c p w -> p c w")
            )
            tg = tgtp.tile([TP, W], I8, tag="tg")
            nc.sync.dma_start(tg[:], tgt.ap()[r0 : r0 + TP, :])

            xs = [xa[:, 0, :], xa[:, 1, :], xa[:, 2, :],
                  xb[:, 0, :], xb[:, 1, :], xb[:, 2, :]]
            pts = []
            prev = xs[0]
            for c in range(1, NCLS):
                pc = ptp.tile([TP, W], F32, tag="pt")
                nc.vector.tensor_max(pc[:], prev, xs[c])
                pts.append(pc)
                prev = pc[:, :]
            m = pts[-1]
            us = []
            for c in range(NCLS - 1):
                u = maskp.tile([TP, W], BF16, tag="u")
                src = xs[0] if c == 0 else pts[c - 1][:, :]
                nc.vector.tensor_tensor(u[:], src, m[:, :], op=OP.is_lt)
                us.append(u)

            fA = fieldp.tile([TP, W + 4], BF16, tag="fA")
            nc.vector.memset(fA[:, 0:2], 0.0)
            nc.vector.memset(fA[:, W + 2 : W + 4], 0.0)
            a1 = maskp.tile([TP, W], BF16, tag="u")
            nc.vector.tensor_add(a1[:], us[0][:, :], us[1][:, :])
            a2 = maskp.tile([TP, W], BF16, tag="u")
            nc.gpsimd.tensor_add(a2[:], us[2][:, :], us[3][:, :])
            a3 = maskp.tile([TP, W], BF16, tag="u")
            nc.vector.tensor_add(a3[:], a1[:, :], a2[:, :])
            nc.gpsimd.tensor_add(fA[:, 2 : W + 2], a3[:, :], us[4][:, :])

            fA2 = fieldp.tile([TP, W + 4], BF16, tag="fA2")
            nc.scalar.activation(fA2[:], fA[:, :], AF.Square)

            fT = fieldp.tile([TP, W + 4], BF16, tag="fT")
            nc.vector.memset(fT[:, 0:2], 0.0)
            nc.vector.memset(fT[:, W + 2 : W + 4], 0.0)
            nc.gpsimd.tensor_copy(fT[:, 2 : W + 2], tg[:, :])
            fT2 = fieldp.tile([TP, W + 4], BF16, tag="fT2")
            nc.scalar.activation(fT2[:], fT[:, :], AF.Square)

            for fi, f in enumerate((fA, fA2, fT, fT2)):
                ws = wsump.tile([TP, W], BF16, tag="ws")
                nc.vector.tensor_add(ws[:], f[:, 0:W], f[:, 1 : W + 1])
                nc.vector.tensor_add(ws[:], ws[:, :], f[:, 2 : W + 2])
                nc.vector.tensor_add(ws[:], ws[:, :], f[:, 3 : W + 3])
                nc.vector.tensor_add(ws[:], ws[:, :], f[:, 4 : W + 4])
                wsums[(fi, j)] = ws

        def phase2(j):
            for h in range(NHALF):
                sl = slice(MMW * h, MMW * (h + 1))
                ps = []
                for fi in range(4):
                    p = psump.tile([TP, MMW], F32, tag="ps")
                    nc.tensor.matmul(
                        p[:], b_m[:], wsums[(fi, j)][:, sl],
                        start=True, stop=j == 0 and j == NT - 1,
                    )
                    if j > 0:
                        nc.tensor.matmul(
                            p[:], b_p[:], wsums[(fi, j - 1)][:, sl],
                            start=False, stop=j == NT - 1,
                        )
                    if j < NT - 1:
                        nc.tensor.matmul(
                            p[:], b_n[:], wsums[(fi, j + 1)][:, sl],
                            start=False, stop=True,
                        )
                    ps.append(p)
                stds = []
                for s1, s2 in ((ps[0], ps[1]), (ps[2], ps[3])):
                    sq = stdp.tile([TP, MMW], F32, tag="sd")
                    nc.scalar.activation(sq[:], s1[:, :], AF.Square, scale=0.2)
                    v = stdp.tile([TP, MMW], F32, tag="sd")
                    nc.vector.tensor_sub(v[:], sq[:, :], s2[:, :])
                    st = stdp.tile([TP, MMW], F32, tag="sd")
                    nc.scalar.activation(st[:], v[:, :], AF.Sqrt, scale=-1.0 / 24.0)
                    stds.append(st)
                d = stdp.tile([TP, MMW], F32, tag="sd")
                nc.vector.tensor_sub(d[:], stds[0][:, :], stds[1][:, :])
                dsq = stdp.tile([TP, MMW], F32, tag="sd")
                slot = 2 * j + h
                nc.scalar.activation(
                    dsq[:], d[:, :], AF.Square, accum_out=acc[:, slot : slot + 1]
                )
            for fi in range(4):
                wsums.pop((fi, j - 1), None)

        for _rep in range(reps):
            for j in range(NT):
                phase1(j)
                if j >= 1:
                    phase2(j - 1)
            phase2(NT - 1)

        accsum = const.tile([128, 1], F32, tag="accsum")
        nc.vector.reduce_sum(accsum[:], acc[:], axis=mybir.AxisListType.X)
        nc.sync.dma_start(out_d.ap(), accsum[:])

    return _legalize_sync_waits(nc) if legalize else nc


def _legalize_sync_waits(nc):
    """This walrus build allows at most 2 sync commands (waits + updates)
    per instruction. Tile emits more on multi-dependency instructions and on
    its kernel-tail drain. Split excess waits onto preceding same-engine
    NoOps (sequencer-executed, so the engine still blocks before the
    original instruction issues — semantics preserved)."""
    import bass_rust

    nid = 0
    for fn in nc.m.functions:
        for blk in fn.blocks:
            il = blk.instructions
            out = []
            changed = False
            for ins in il:
                si = ins.sync_info
                if si is not None:
                    waits = list(si.on_wait)
                    upds = list(si.on_update)
                    keep = 1
                    if len(waits) > keep:
                        extra = waits[: len(waits) - keep]
                        kept = waits[len(waits) - keep :]
                        for w in extra:
                            nop = mybir.InstNoOp(
                                name=f"syncsplit-{nid}", ins=[], outs=[]
                            )
                            nid += 1
                            nop.engine = ins.engine
                            nop.sync_info = bass_rust.SyncInfo(
                                on_wait=[w], on_update=[]
                            )
                            out.append(nop)
                        ins.sync_info = bass_rust.SyncInfo(
                            on_wait=kept, on_update=upds
                        )
                        changed = True
                out.append(ins)
            if changed:
                il.clear()
                il.extend(out)
    return nc


_NC_CACHE = {}


def _get_nc(H, W, reps=1):
    key = (H, W, reps)
    if key not in _NC_CACHE:
        _NC_CACHE[key] = build_nc(H, W, reps=reps)
    return _NC_CACHE[key]


def make_in_maps(predictions, targets):
    B = predictions.shape[0]
    t8 = np.ascontiguousarray(targets.astype(np.int8))
    return [
        {"pred": np.ascontiguousarray(predictions[b]), "tgt": t8[b]}
        for b in range(B)
    ]


def kernel(predictions, targets, kernel_size):
    from concourse.bass_utils import run_bass_kernel_spmd

    assert int(kernel_size) == 5
    predictions = np.asarray(predictions)
    targets = np.asarray(targets)
    B, C, H, W = predictions.shape
    assert C == NCLS and B == N_CORES

    nc = _get_nc(H, W)
    in_maps = make_in_maps(predictions, targets)
    res = run_bass_kernel_spmd(nc, in_maps, core_ids=list(range(N_CORES)))
    total = 0.0
    for r in res.results:
        total += float(r["out"].astype(np.float64).sum())
    return np.float32(total / (B * H * W))


# revision 12
# speedup vs baseline: 536.4876x; 1.1768x over previous
"""Trainium2 Bass kernel: windowed-std consistency loss.

Computes mean((local_std(argmax_c pred) - local_std(tgt))^2) with 5x5
zero-padded windows and unbiased (n-1) variance, n fixed at 25.

Per core (one batch element), two interleaved phases over 8 aligned
128-row tiles:

  phase 1 (per tile): DMA 6 class planes (f32, contiguous 4KB lines) +
      int8 targets; exact argmax A via monotone prefix-max masks
      (5 max + 5 is_lt on DVE); fields A, A^2, T, T^2 as zero-padded
      [128, W+4] bf16 tiles; W-direction 5-box-sum via 3 bf16 adds per
      field on DVE (pair-sum b2 = f0+f1, then ws = b2 + b2<<2 + f4;
      partial sums <= 250, exact in bf16; in-place accumulate is safe,
      same element offsets).
  phase 2 (per tile, lagged by 1): H-direction 5-box-sum on PE — per
      field a banded [128,128] matmul plus zero-padded seam bands that
      accumulate the +-2-row contributions from the adjacent tiles'
      W-sums into the same PSUM group (start/stop accumulation), so
      tiles stay 128-aligned and nothing is re-fetched or recomputed.
      var/std straight out of PSUM: s1q = Square(0.2*s1) [Act],
      v = s1q - s2 [DVE] (= -24 var), std = Sqrt(-v/24) [Act],
      d = stdA - stdT [DVE], Square+accum into acc column [Act].

Targets ship as int8 (values 0..5; lossless cast from int64/int32 on
host), cutting per-core input bytes from 32MB to 25MB.
"""

from contextlib import ExitStack

import ml_dtypes
import numpy as np

import concourse.bass as bass
import concourse.tile as tile
from concourse import mybir

F32 = mybir.dt.float32
BF16 = mybir.dt.bfloat16
I8 = mybir.dt.int8
AF = mybir.ActivationFunctionType
OP = mybir.AluOpType

NCLS = 6
N_CORES = 8
TP = 128  # tile rows (partition dim)


def _bands_np():
    """Main/prev/next H-direction 5-band blocks, bf16 [128,128].

    out[h'] = sum_h band[h, h'] * x[h]; main covers |h-h'|<=2 within the
    tile, prev/next cover the 2-row halos from the adjacent tiles.
    """
    r = np.arange(TP)[:, None]
    c = np.arange(TP)[None, :]
    main = (np.abs(r - c) <= 2).astype(np.float32)
    prev = ((r >= TP - 2) & (np.abs((r - TP) - c) <= 2)).astype(np.float32)
    nxt = ((r <= 1) & (np.abs((r + TP) - c) <= 2)).astype(np.float32)
    return [b.astype(ml_dtypes.bfloat16) for b in (main, prev, nxt)]


def build_nc(H, W, reps=1, legalize=True):
    assert H % TP == 0
    NT = H // TP
    nc = bass.Bass()
    pred = nc.dram_tensor("pred", [NCLS, H, W], F32, kind="ExternalInput")
    tgt = nc.dram_tensor("tgt", [H, W], I8, kind="ExternalInput")
    out_d = nc.dram_tensor("out", [128, 1], F32, kind="ExternalOutput")

    bm_np, bp_np, bn_np = _bands_np()
    bm_d = nc.inline_tensor(np.ascontiguousarray(bm_np), "band_m")
    bp_d = nc.inline_tensor(np.ascontiguousarray(bp_np), "band_p")
    bn_d = nc.inline_tensor(np.ascontiguousarray(bn_np), "band_n")

    MMW = 512  # matmul free-dim chunk (one PSUM bank of f32)
    NHALF = W // MMW

    with tile.TileContext(nc) as tc, ExitStack() as ctx:
        const = ctx.enter_context(tc.tile_pool(name="const", bufs=1))
        predp = ctx.enter_context(tc.tile_pool(name="predp", bufs=3))
        tgtp = ctx.enter_context(tc.tile_pool(name="tgtp", bufs=2))
        ptp = ctx.enter_context(tc.tile_pool(name="ptp", bufs=5))
        maskp = ctx.enter_context(tc.tile_pool(name="maskp", bufs=6))
        fieldp = ctx.enter_context(tc.tile_pool(name="fieldp", bufs=6))
        wtmpp = ctx.enter_context(tc.tile_pool(name="wtmpp", bufs=4))
        wsump = ctx.enter_context(tc.tile_pool(name="wsump", bufs=12))
        stdp = ctx.enter_context(tc.tile_pool(name="stdp", bufs=4))
        psump = ctx.enter_context(tc.tile_pool(name="psump", bufs=8, space="PSUM"))

        b_m = const.tile([TP, TP], BF16, tag="bm")
        nc.sync.dma_start(b_m[:], bm_d.ap())
        b_p = const.tile([TP, TP], BF16, tag="bp")
        nc.sync.dma_start(b_p[:], bp_d.ap())
        b_n = const.tile([TP, TP], BF16, tag="bn")
        nc.sync.dma_start(b_n[:], bn_d.ap())

        acc = const.tile([128, 2 * NT], F32, tag="acc")
        nc.vector.memset(acc[:], 0.0)

        wsums = {}  # (field, tile_j) -> [128, W] bf16 W-direction box sums

        def phase1(j):
            r0 = TP * j
            xa = predp.tile([TP, 3, W], F32, tag="xa")
            nc.sync.dma_start(
                xa[:], pred.ap()[0:3, r0 : r0 + TP, :].rearrange("c p w -> p c w")
            )
            xb = predp.tile([TP, 3, W], F32, tag="xb")
            nc.scalar.dma_start(
                xb[:], pred.ap()[3:6, r0 : r0 + TP, :].rearrange("c p w -> p c w")
            )
            tg = tgtp.tile([TP, W], I8, tag="tg")
            nc.sync.dma_start(tg[:], tgt.ap()[r0 : r0 + TP, :])

            xs = [xa[:, 0, :], xa[:, 1, :], xa[:, 2, :],
                  xb[:, 0, :], xb[:, 1, :], xb[:, 2, :]]
            pts = []
            prev = xs[0]
            for c in range(1, NCLS):
                pc = ptp.tile([TP, W], F32, tag="pt")
                nc.vector.tensor_max(pc[:], prev, xs[c])
                pts.append(pc)
                prev = pc[:, :]
            m = pts[-1]
            us = []
            for c in range(NCLS - 1):
                u = maskp.tile([TP, W], BF16, tag="u")
                src = xs[0] if c == 0 else pts[c - 1][:, :]
                nc.vector.tensor_tensor(u[:], src, m[:, :], op=OP.is_lt)
                us.append(u)

            fA = fieldp.tile([TP, W + 4], BF16, tag="fA")
            nc.vector.memset(fA[:, 0:2], 0.0)
            nc.vector.memset(fA[:, W + 2 : W + 4], 0.0)
            a1 = maskp.tile([TP, W], BF16, tag="u")
            nc.vector.tensor_add(a1[:], us[0][:, :], us[1][:, :])
            a2 = maskp.tile([TP, W], BF16, tag="u")
            nc.gpsimd.tensor_add(a2[:], us[2][:, :], us[3][:, :])
            a3 = maskp.tile([TP, W], BF16, tag="u")
            nc.vector.tensor_add(a3[:], a1[:, :], a2[:, :])
            nc.gpsimd.tensor_add(fA[:, 2 : W + 2], a3[:, :], us[4][:, :])

            fA2 = fieldp.tile([TP, W + 4], BF16, tag="fA2")
            nc.scalar.activation(fA2[:], fA[:, :], AF.Square)

            fT = fieldp.tile([TP, W + 4], BF16, tag="fT")
            nc.vector.memset(fT[:, 0:2], 0.0)
            nc.vector.memset(fT[:, W + 2 : W + 4], 0.0)
            nc.gpsimd.tensor_copy(fT[:, 2 : W + 2], tg[:, :])
            fT2 = fieldp.tile([TP, W + 4], BF16, tag="fT2")
            nc.scalar.activation(fT2[:], fT[:, :], AF.Square)

            for fi, f in enumerate((fA, fA2, fT, fT2)):
                # 5-tap box in 3 adds: b2[w] = f[w]+f[w+1];
                # ws[w] = b2[w] + b2[w+2] + f[w+4]
                b2 = wtmpp.tile([TP, W + 3], BF16, tag="b2")
                nc.vector.tensor_add(b2[:], f[:, 0 : W + 3], f[:, 1 : W + 4])
                ws = wsump.tile([TP, W], BF16, tag="ws")
                nc.vector.tensor_add(ws[:], b2[:, 0:W], b2[:, 2 : W + 2])
                nc.vector.tensor_add(ws[:], ws[:, :], f[:, 4 : W + 4])
                wsums[(fi, j)] = ws

        def phase2(j):
            for h in range(NHALF):
                sl = slice(MMW * h, MMW * (h + 1))
                ps = []
                for fi in range(4):
                    p = psump.tile([TP, MMW], F32, tag="ps")
                    nc.tensor.matmul(
                        p[:], b_m[:], wsums[(fi, j)][:, sl],
                        start=True, stop=j == 0 and j == NT - 1,
                    )
                    if j > 0:
                        nc.tensor.matmul(
                            p[:], b_p[:], wsums[(fi, j - 1)][:, sl],
                            start=False, stop=j == NT - 1,
                        )
                    if j < NT - 1:
                        nc.tensor.matmul(
                            p[:], b_n[:], wsums[(fi, j + 1)][:, sl],
                            start=False, stop=True,
                        )
                    ps.append(p)
                stds = []
                for s1, s2 in ((ps[0], ps[1]), (ps[2], ps[3])):
                    sq = stdp.tile([TP, MMW], F32, tag="sd")
                    nc.scalar.activation(sq[:], s1[:, :], AF.Square, scale=0.2)
                    v = stdp.tile([TP, MMW], F32, tag="sd")
                    nc.vector.tensor_sub(v[:], sq[:, :], s2[:, :])
                    st = stdp.tile([TP, MMW], F32, tag="sd")
                    nc.scalar.activation(st[:], v[:, :], AF.Sqrt, scale=-1.0 / 24.0)
                    stds.append(st)
                d = stdp.tile([TP, MMW], F32, tag="sd")
                nc.vector.tensor_sub(d[:], stds[0][:, :], stds[1][:, :])
                dsq = stdp.tile([TP, MMW], F32, tag="sd")
                slot = 2 * j + h
                nc.scalar.activation(
                    dsq[:], d[:, :], AF.Square, accum_out=acc[:, slot : slot + 1]
                )
            for fi in range(4):
                wsums.pop((fi, j - 1), None)

        for _rep in range(reps):
            for j in range(NT):
                phase1(j)
                if j >= 1:
                    phase2(j - 1)
            phase2(NT - 1)

        accsum = const.tile([128, 1], F32, tag="accsum")
        nc.vector.reduce_sum(accsum[:], acc[:], axis=mybir.AxisListType.X)
        nc.sync.dma_start(out_d.ap(), accsum[:])

    return _legalize_sync_waits(nc) if legalize else nc


def _legalize_sync_waits(nc):
    """This walrus build allows at most 2 sync commands (waits + updates)
    per instruction. Tile emits more on multi-dependency instructions and on
    its kernel-tail drain. Split excess waits onto preceding same-engine
    NoOps (sequencer-executed, so the engine still blocks before the
    original instruction issues — semantics preserved)."""
    import bass_rust

    nid = 0
    for fn in nc.m.functions:
        for blk in fn.blocks:
            il = blk.instructions
            out = []
            changed = False
            for ins in il:
                si = ins.sync_info
                if si is not None:
                    waits = list(si.on_wait)
                    upds = list(si.on_update)
                    keep = 1
                    if len(waits) > keep:
                        extra = waits[: len(waits) - keep]
                        kept = waits[len(waits) - keep :]
                        for w in extra:
                            nop = mybir.InstNoOp(
                                name=f"syncsplit-{nid}", ins=[], outs=[]
                            )
                            nid += 1
                            nop.engine = ins.engine
                            nop.sync_info = bass_rust.SyncInfo(
                                on_wait=[w], on_update=[]
                            )
                            out.append(nop)
                        ins.sync_info = bass_rust.SyncInfo(
                            on_wait=kept, on_update=upds
                        )
                        changed = True
                out.append(ins)
            if changed:
                il.clear()
                il.extend(out)
    return nc


_NC_CACHE = {}


def _get_nc(H, W, reps=1):
    key = (H, W, reps)
    if key not in _NC_CACHE:
        _NC_CACHE[key] = build_nc(H, W, reps=reps)
    return _NC_CACHE[key]


def make_in_maps(predictions, targets):
    B = predictions.shape[0]
    t8 = np.ascontiguousarray(targets.astype(np.int8))
    return [
        {"pred": np.ascontiguousarray(predictions[b]), "tgt": t8[b]}
        for b in range(B)
    ]


def kernel(predictions, targets, kernel_size):
    from concourse.bass_utils import run_bass_kernel_spmd

    assert int(kernel_size) == 5
    predictions = np.asarray(predictions)
    targets = np.asarray(targets)
    B, C, H, W = predictions.shape
    assert C == NCLS and B == N_CORES

    nc = _get_nc(H, W)
    in_maps = make_in_maps(predictions, targets)
    res = run_bass_kernel_spmd(nc, in_maps, core_ids=list(range(N_CORES)))
    total = 0.0
    for r in res.results:
        total += float(r["out"].astype(np.float64).sum())
    return np.float32(total / (B * H * W))
